# revision 51
# baseline (speedup 1.0000x reference)
"""AffCoeffToMatrix TRN2 kernel (v8: fp16 planar I/O, pipelined groups).

For each batch element (B = 2,000,000):
  R = rodrigues(rotat), U = rodrigues(scal_dir), D = exp(scal)
  M = R @ (U @ diag(D) @ U^T);  out = [M | trans]  -> [B, 3, 4] f32

Host marshals inputs to fp16 PLANAR layout (9 planes: r_xyz, u_xyz, s_xyz)
and reassembles the full [B,3,4] f32 output from the 9 fp16 M-planes the
device returns, inserting the trans column exactly.  Device HBM traffic is
36 B/elem (18 in + 18 out) vs 96 B/elem for f32 interleaved full I/O, and
all DMAs move >=984 B contiguous runs (full-rate).

On-core: L = 1968 elems/lane split into variable-width groups
(WIDTHS=(1,3,2,2) x F=246; narrow first group fills the pipeline fast).
Per group: ACT chain (squares, Ln/Exp in natural_log_exp table, Sin pair in
trig table) then per-246-wide-block builds.  Chain tiles with bufs=1 force
clean cross-group table phasing via WAR deps: exactly 2 table loads per
group.  cos(th/2) = sin(pi/2 - th/2) needs no range fold for th < 3pi.

Math per rotation (v = axis vector, th2 = |v|^2):
  lg = ln(th2); th = e^{lg/2}; rt = e^{-lg/2 + ln sqrt2} = sqrt2/th
  sh = sin(th/2), ch = cos(th/2); t2 = sh*rt; G = t2*v
  c2 = 1 - 2 sh^2 = cos th,  C2 = sqrt2*ch
  R = c2 I + G G^T + [C2 G]x          (G_i G_j = b v_i v_j, C2 G_k = a v_k)
Scaling: W = U diag(e^{s/2}), S = W W^T (6 unique), M = R @ S.

Engines (v1 cost model): ACT squares + transcendentals + most dG/sqW, DVE
fp16 2x tensor muls + 4x tensor-scalar coefficients, Pool assembly adds +
final M adds, SP all DMAs.  Drain: last block's M add and the final out
DMA are split per row-plane so the tail ships early.
"""
import math
import sys

for _p in ("/opt/trn_rl_repo", "/root/.axon_site/_ro/trn_rl_repo"):
    if _p not in sys.path:
        sys.path.append(_p)

import numpy as np

import concourse.bass as bass
import concourse.mybir as mybir
import concourse.tile as tile

F32 = mybir.dt.float32
F16 = mybir.dt.float16
AF = mybir.ActivationFunctionType
OP = mybir.AluOpType
PI = math.pi

# ---- hardcoded problem geometry ----
B = 2_000_000
N_CORES = 8
P = 128
F = 246            # base tile width (2F chunks = 984B >= 512B DMA full rate)
F2 = 2 * F         # group width (chains and builds)
NQ = 4             # groups per sweep
L = F2 * NQ        # elems per partition lane (1968)
E = P * L          # elems per core (251904)
BPAD = N_CORES * E


def _split_multi_waits(nc, limit=1, drain_limit=0):
    """This container's walrus cannot encode >1 sync-wait per instruction
    (Drain: none at all). Spill extras onto same-engine NOPs."""
    for b in nc.main_func.blocks:
        new = []
        for ins in b.instructions:
            si = getattr(ins, "sync_info", None)
            waits = list(si.on_wait) if (si is not None and si.on_wait) else []
            lim = drain_limit if isinstance(ins, mybir.InstDrain) else limit
            if len(waits) > lim:
                keep, spill = waits[:lim], waits[lim:]
                for w in spill:
                    nop = mybir.InstNoOp(
                        name=nc.get_next_instruction_name(),
                        sync_info=mybir.SyncInfo(on_wait=[w], on_update=[]),
                        bass_nofuse=True,
                        engine=ins.engine,
                    )
                    nc.register_instruction(nop)
                    new.append(nop)
                ins.sync_info = mybir.SyncInfo(
                    on_wait=keep, on_update=list(si.on_update or [])
                )
            new.append(ins)
        b.instructions[:] = new


def build_module(WIDTHS=(1, 3, 2, 2), CLUSTERS=None, DG_POOL=1, MS_DVE=1, SQW_DVE_EARLY=1, DG_DVE_EARLY=0, MP2_POOL_LATE=7, ROWSPLIT_FROM=7, S_DVE_LATE=5, OT_DVE_LATE=7, HALF_TAIL=False, BLD=2, RU=2, PMAT_BUFS=2, PMS_BUFS=2, SMALL=2, OUT=2):
    nc = bass.Bass()
    in9 = nc.dram_tensor("in9", [9, E], F16, kind="ExternalInput")
    out9 = nc.dram_tensor("out9", [9, E], F16, kind="ExternalOutput")

    F1 = F
    assert sum(WIDTHS) == 8
    GQMAX = max(WIDTHS) * F1
    starts = [sum(WIDTHS[:i]) * F1 for i in range(len(WIDTHS))]

    vinL = in9[:].rearrange("k (p f) -> p k f", p=P)    # [P, 9, L]
    voutL = out9[:].rearrange("k (p f) -> p k f", p=P)  # [P, 9, L]

    with tile.TileContext(nc) as tc:
        with (
            tc.tile_pool(name="pin", bufs=2) as pin,      # in36 group
            tc.tile_pool(name="pth2", bufs=2) as pth2,    # th2 f32 (Pool writes)
            tc.tile_pool(name="ppsum", bufs=2, space="PSUM") as ppsum,  # lg f32
            tc.tile_pool(name="pch", bufs=1) as pch,      # chain transients (bufs=1 also forces cross-group ACT table phasing via WAR deps)
            tc.tile_pool(name="pcf", bufs=2) as pcf,      # t2, C2, c2 survivors
            tc.tile_pool(name="pe3", bufs=2) as pe3,      # e3
            tc.tile_pool(name="psq", bufs=1) as psq,      # squares scratch
            tc.tile_pool(name="pbld", bufs=BLD) as pbld,  # G/dG/av/p6 (F1 blocks)
            tc.tile_pool(name="pru", bufs=RU) as pru,     # RU18
            tc.tile_pool(name="pmat", bufs=PMAT_BUFS) as pmat,  # W9/sqW/S9/pp
            tc.tile_pool(name="pms", bufs=PMS_BUFS) as pms,     # ms
            tc.tile_pool(name="psmall", bufs=SMALL) as psmall,  # sdt/q3
            tc.tile_pool(name="pout", bufs=OUT) as pout,  # out per group
            tc.tile_pool(name="pc", bufs=1) as pc,
        ):
            # const bias tiles + dummy Ln to warm the natural_log_exp table
            # during the first DMA
            lnr2 = pc.tile([P, 1], F32, tag="lnr2")
            nc.vector.memset(lnr2[:], 0.5 * math.log(2.0))
            pi2 = pc.tile([P, 1], F32, tag="pi2")
            nc.vector.memset(pi2[:], PI / 2)
            warm1 = pc.tile([P, 1], F32, tag="warm1")
            nc.scalar.activation(warm1[:], pi2[:], AF.Ln)

            def chain_natlog(q, slot):
                w = WIDTHS[q]
                GQ = w * F1
                gsl = slice(starts[q], starts[q] + GQ)
                in36 = pin.tile([P, 9 * GQMAX], F16, tag=f"in36{slot}", name="in36")
                v36 = in36[:].rearrange("p (k f) -> p k f", k=9)[:, :, :GQ]
                if q <= 1:
                    # split early DMAs so squares start sooner
                    nc.sync.dma_start(out=v36[:, 0:6, :], in_=vinL[:, 0:6, gsl])
                    nc.sync.dma_start(out=v36[:, 6:9, :], in_=vinL[:, 6:9, gsl])
                else:
                    nc.sync.dma_start(out=v36, in_=vinL[:, :, gsl])

                def t16(pool, n, tag):
                    full = pool.tile([P, n * GQMAX], F16, tag=tag + str(slot), name=tag)
                    return full[:].rearrange("p (r f) -> p r f", r=n)[:, :, :GQ]

                th2f = pth2.tile([P, 2 * GQMAX], F32, tag="th2", name="th2")
                th2v = th2f[:].rearrange("p (r f) -> p r f", r=2)[:, :, :GQ]
                lgf = ppsum.tile([P, 2 * GQMAX], F32, tag="lg", name="lg")
                lgv = lgf[:].rearrange("p (r f) -> p r f", r=2)[:, :, :GQ]
                thv = t16(pch, 2, "th")
                rtv = t16(pch, 2, "rt")
                e3v = t16(pe3, 3, "e3")
                shv = t16(pch, 2, "sh")
                chv = t16(pch, 2, "ch")
                shsqv = t16(pch, 2, "shsq")
                t2v = t16(pcf, 2, "t2")
                C2v = t16(pcf, 2, "C2")
                c2v = t16(pcf, 2, "c2")

                def hs2(v, h):
                    return v[:, :, h * F1 : (h + 1) * F1]

                # squares + th2 + natlog phase: split per F1-half for
                # latency on the first two groups, full-width after (the
                # pipeline is full; fewer ACT instructions win)
                subs = [None] if (q > 1 and w <= 2) else range(w)
                for h in subs:
                    if h is None:
                        sl = slice(0, GQ)
                        wq = GQ
                    else:
                        sl = slice(h * F1, (h + 1) * F1)
                        wq = F1
                    sq = psq.tile([P, 6 * 2 * F1], F16, tag="sq", name="sq")
                    sqv = sq[:].rearrange("p (c f) -> p c f", c=6)[:, :, :wq]
                    nc.scalar.activation(sqv, v36[:, 0:6, sl], AF.Square)
                    tmp = psq.tile([P, 2 * 2 * F1], F16, tag="tmp", name="tmp")
                    tmpv = tmp[:].rearrange("p (r f) -> p r f", r=2)[:, :, :wq]
                    nc.gpsimd.tensor_add(tmpv, sqv[:, 0:4:3, :], sqv[:, 1:5:3, :])
                    nc.gpsimd.tensor_add(th2v[:, :, sl], tmpv, sqv[:, 2:6:3, :])
                    nc.scalar.activation(lgv[:, :, sl], th2v[:, :, sl], AF.Ln)
                    nc.scalar.activation(
                        thv[:, :, sl], lgv[:, :, sl], AF.Exp, scale=0.5
                    )
                    nc.scalar.activation(
                        rtv[:, :, sl], lgv[:, :, sl], AF.Exp, scale=-0.5,
                        bias=lnr2[:],
                    )
                    nc.scalar.activation(
                        e3v[:, :, sl], v36[:, 6:9, sl], AF.Exp, scale=0.5
                    )

                return {
                    "v36": v36, "th": thv, "rt": rtv, "e3": e3v,
                    "GQ": GQ,
                }

            def chain_trig(q, slot, stn):
                GQ = stn["GQ"]
                thv, rtv = stn["th"], stn["rt"]

                def t16(pool, n, tag):
                    full = pool.tile([P, n * GQMAX], F16, tag=tag + str(slot), name=tag)
                    return full[:].rearrange("p (r f) -> p r f", r=n)[:, :, :GQ]

                sh = t16(pch, 2, "sh")
                ch = t16(pch, 2, "ch")
                shsq = t16(pch, 2, "shsq")
                t2v = t16(pcf, 2, "t2")
                C2v = t16(pcf, 2, "C2")
                c2v = t16(pcf, 2, "c2")
                # cos(x) = sin(pi/2 - x): arg in [-1.44, pi/2] for th < 3pi,
                # always inside the Sin table domain -> no range fold needed.
                # Split per F1-half on group 1 so its first build starts
                # sooner (pch WAR deps still pin cross-group table phases).
                w = WIDTHS[q]
                tsubs = [None]
                for h in tsubs:
                    tsl = slice(0, GQ) if h is None else slice(h * F1, (h + 1) * F1)
                    nc.scalar.activation(
                        sh[:, :, tsl], thv[:, :, tsl], AF.Sin, scale=0.5
                    )
                    nc.scalar.activation(
                        ch[:, :, tsl], thv[:, :, tsl], AF.Sin, scale=-0.5,
                        bias=pi2[:],
                    )
                    nc.scalar.activation(
                        shsq[:, :, tsl], sh[:, :, tsl], AF.Square
                    )
                    nc.vector.tensor_mul(
                        t2v[:, :, tsl], sh[:, :, tsl], rtv[:, :, tsl]
                    )
                    nc.vector.tensor_scalar(
                        C2v[:, :, tsl], ch[:, :, tsl], math.sqrt(2.0), None,
                        OP.mult,
                    )
                    nc.vector.tensor_scalar(
                        c2v[:, :, tsl], shsq[:, :, tsl], -2.0, 1.0, OP.mult,
                        OP.add,
                    )
                return {
                    "v36": stn["v36"],
                    "t2": t2v,
                    "C2": C2v,
                    "c2": c2v,
                    "e3": stn["e3"],
                }

            BCTR = [0]

            def build(st, foff, bw, ot, osl, tagsfx=""):
                bidx = BCTR[0]
                BCTR[0] += 1
                """One bw-wide build block; writes M into ot[:, :, osl]."""
                sl = slice(foff, foff + bw)
                vv = st["v36"][:, 0:6, sl].rearrange("p (r c) f -> p r c f", r=2)
                t2s = st["t2"][:, :, sl]
                C2s = st["C2"][:, :, sl].unsqueeze(2)
                c2s = st["c2"][:, :, sl]
                e3s = st["e3"][:, :, sl]

                # G = t2 * v
                G = pbld.tile([P, 6 * bw], F16, tag=tagsfx + "G", name="G")
                Gv = G[:].rearrange("p (r c f) -> p r c f", r=2, c=3)
                nc.vector.tensor_mul(
                    Gv, t2s.unsqueeze(2).to_broadcast((P, 2, 3, bw)), vv
                )
                # dG = G^2 (ACT Square, or Pool mul on DG_POOL blocks)
                dG = pbld.tile([P, 6 * bw], F16, tag=tagsfx + "dG", name="dG")
                dGv = dG[:].rearrange("p (r c f) -> p r c f", r=2, c=3)
                if bidx < DG_DVE_EARLY:
                    nc.vector.tensor_mul(dGv, Gv, Gv)
                elif bidx < DG_DVE_EARLY + DG_POOL:
                    nc.gpsimd.tensor_mul(dGv, Gv, Gv)
                else:
                    nc.scalar.activation(dGv, Gv, AF.Square)
                # av planes in (z, x, y) order: av = C2 * G_perm
                av = pbld.tile([P, 6 * bw], F16, tag=tagsfx + "av", name="av")
                avv = av[:].rearrange("p (r c f) -> p r c f", r=2, c=3)
                nc.vector.tensor_mul(
                    avv[:, :, 0:1, :],
                    C2s.to_broadcast((P, 2, 1, bw)),
                    Gv[:, :, 2:3, :],
                )
                nc.vector.tensor_mul(
                    avv[:, :, 1:3, :],
                    C2s.to_broadcast((P, 2, 2, bw)),
                    Gv[:, :, 0:2, :],
                )
                # p6 = (G0G1, G1G2, G2G0)
                p6 = pbld.tile([P, 6 * bw], F16, tag=tagsfx + "p6", name="p6")
                p6v = p6[:].rearrange("p (r c f) -> p r c f", r=2, c=3)
                nc.vector.tensor_mul(
                    p6v[:, :, 0:2, :], Gv[:, :, 0:2, :], Gv[:, :, 1:3, :]
                )
                nc.vector.tensor_mul(
                    p6v[:, :, 2:3, :], Gv[:, :, 2:3, :], Gv[:, :, 0:1, :]
                )

                # RU18 assembly (Pool): R = c2 I + GG^T + [C2 G]x
                RU18 = pru.tile([P, 18 * bw], F16, tag=tagsfx + "RU18", name="RU18")
                ruv = RU18[:].rearrange("p (r k f) -> p r k f", r=2, k=9)
                c2b = c2s.unsqueeze(2).to_broadcast((P, 2, 3, bw))
                nc.gpsimd.tensor_add(ruv[:, :, 0:9:4, :], dGv, c2b)
                nc.gpsimd.tensor_add(
                    ruv[:, :, 3:8:4, :], p6v[:, :, 0:2, :], avv[:, :, 0:2, :]
                )
                nc.gpsimd.tensor_add(
                    ruv[:, :, 2, :], p6v[:, :, 2, :], avv[:, :, 2, :]
                )
                nc.gpsimd.tensor_sub(
                    ruv[:, :, 1:6:4, :], p6v[:, :, 0:2, :], avv[:, :, 0:2, :]
                )
                nc.gpsimd.tensor_sub(
                    ruv[:, :, 6, :], p6v[:, :, 2, :], avv[:, :, 2, :]
                )

                R9v = RU18[:, : 9 * bw].rearrange("p (k f) -> p k f", k=9)
                U9v = RU18[:, 9 * bw :].rearrange("p (i k f) -> p i k f", i=3, k=3)

                # W = U diag(e) (DVE), sqW (ACT)
                W9 = pmat.tile([P, 9 * bw], F16, tag=tagsfx + "W9", name="W9")
                W9v4 = W9[:].rearrange("p (i k f) -> p i k f", i=3, k=3)
                nc.vector.tensor_mul(
                    W9v4, U9v, e3s.unsqueeze(1).to_broadcast((P, 3, 3, bw))
                )
                sqW = pmat.tile([P, 9 * bw], F16, tag=tagsfx + "sqW", name="sqW")
                if bidx < SQW_DVE_EARLY:
                    nc.vector.tensor_mul(sqW[:], W9[:], W9[:])
                else:
                    nc.scalar.activation(sqW[:], W9[:], AF.Square)
                sqWv = sqW[:].rearrange("p (i k f) -> p i k f", i=3, k=3)

                # S unique-6: S00@0 S01@1 S02@2 S11@3 S12@5 S22@8
                S9 = pmat.tile([P, 9 * bw], F16, tag=tagsfx + "S9", name="S9")
                S9v = S9[:].rearrange("p (k f) -> p k f", k=9)
                sdt = psmall.tile([P, 3 * bw], F16, tag=tagsfx + "sdt", name="sdt")
                sdtv = sdt[:].rearrange("p (c f) -> p c f", c=3)
                nc.gpsimd.tensor_add(sdtv, sqWv[:, :, 0, :], sqWv[:, :, 1, :])
                nc.gpsimd.tensor_add(
                    S9v[:, 0:4:3, :], sdtv[:, 0:2, :], sqWv[:, 0:2, 2, :]
                )
                nc.gpsimd.tensor_add(S9v[:, 8, :], sdtv[:, 2, :], sqWv[:, 2, 2, :])
                # pp: row-pair products (01, 02, 12)
                pp = pmat.tile([P, 9 * bw], F16, tag=tagsfx + "pp", name="pp")
                ppv = pp[:].rearrange("p (g k f) -> p g k f", g=3, k=3)
                nc.vector.tensor_mul(
                    ppv[:, 0:2, :, :],
                    W9v4[:, 0, :, :].unsqueeze(1).to_broadcast((P, 2, 3, bw)),
                    W9v4[:, 1:3, :, :],
                )
                nc.vector.tensor_mul(
                    ppv[:, 2, :, :], W9v4[:, 1, :, :], W9v4[:, 2, :, :]
                )
                q3 = psmall.tile([P, 3 * bw], F16, tag=tagsfx + "q3", name="q3")
                q3v = q3[:].rearrange("p (g f) -> p g f", g=3)
                seng = nc.vector if bidx >= S_DVE_LATE else nc.gpsimd
                seng.tensor_add(q3v, ppv[:, :, 0, :], ppv[:, :, 1, :])
                seng.tensor_add(
                    S9v[:, 1:3, :], q3v[:, 0:2, :], ppv[:, 0:2, 2, :]
                )
                seng.tensor_add(S9v[:, 5, :], q3v[:, 2, :], ppv[:, 2, 2, :])

                # M = R @ S (DVE muls, Pool final add into out tile)
                srows = [S9v[:, 0:3, :], S9v[:, 1:7:2, :], S9v[:, 2:9:3, :]]
                otv = ot.rearrange("p (i j) f -> p i j f", i=3)[:, :, :, osl]

                def colb(k):
                    return (
                        R9v[:, k : k + 7 : 3, :]
                        .unsqueeze(2)
                        .to_broadcast((P, 3, 3, bw))
                    )

                def rowb(sr):
                    return sr.unsqueeze(1).to_broadcast((P, 3, 3, bw))

                mp1 = pmat.tile([P, 9 * bw], F16, tag=tagsfx + "pp", name="mp1")
                mp1v = mp1[:].rearrange("p (i j f) -> p i j f", i=3, j=3)
                nc.vector.tensor_mul(mp1v, colb(0), rowb(srows[0]))
                mp2 = pmat.tile([P, 9 * bw], F16, tag=tagsfx + "sqW", name="mp2")
                mp2v = mp2[:].rearrange("p (i j f) -> p i j f", i=3, j=3)
                (nc.gpsimd if bidx >= MP2_POOL_LATE else nc.vector).tensor_mul(
                    mp2v, colb(1), rowb(srows[1])
                )
                ms = pms.tile([P, 9 * bw], F16, tag=tagsfx + "ms", name="ms")
                msv = ms[:].rearrange("p (i j f) -> p i j f", i=3, j=3)
                (nc.vector if bidx >= 8 - MS_DVE else nc.gpsimd).tensor_add(
                    msv, mp1v, mp2v
                )
                mp3 = pmat.tile([P, 9 * bw], F16, tag=tagsfx + "W9", name="mp3")
                mp3v = mp3[:].rearrange("p (i j f) -> p i j f", i=3, j=3)
                nc.vector.tensor_mul(mp3v, colb(2), rowb(srows[2]))
                oeng = nc.vector if bidx >= OT_DVE_LATE else nc.gpsimd
                if bidx >= ROWSPLIT_FROM:
                    # drain: per-row adds so output planes ship early
                    for i in range(3):
                        oeng.tensor_add(
                            otv[:, i : i + 1, :, :],
                            msv[:, i : i + 1, :, :],
                            mp3v[:, i : i + 1, :, :],
                        )
                else:
                    oeng.tensor_add(otv, msv, mp3v)

            if CLUSTERS is None:
                CLUSTERS2 = tuple((i,) for i in range(len(WIDTHS)))
            else:
                CLUSTERS2 = CLUSTERS
            for cluster in CLUSTERS2:
              cl_states = {}
              for slot, q in enumerate(cluster):
                  cl_states[q] = chain_natlog(q, slot)
              for slot, q in enumerate(cluster):
                  cl_states[q] = chain_trig(q, slot, cl_states[q])
              for slot, q in enumerate(cluster):
                w = WIDTHS[q]
                GQ = w * F1
                gsl = slice(starts[q], starts[q] + GQ)
                st = cl_states[q]
                is_last_group = q == len(WIDTHS) - 1
                # fixed-size 2-block output chunks (keeps pout independent
                # of group width; odd tail chunk pays the <512B DMA rate)
                b = 0
                while b < w:
                    cw = min(2, w - b)
                    otf = pout.tile([P, 9 * 2 * F1], F16, tag="ot", name="ot")
                    otk = otf[:].rearrange("p (k f) -> p k f", k=9)[
                        :, :, : cw * F1
                    ]
                    for j in range(cw):
                        build(st, (b + j) * F1, F1, otk,
                              slice(j * F1, (j + 1) * F1))
                    dsl = slice(starts[q] + b * F1, starts[q] + (b + cw) * F1)
                    if is_last_group and b + cw == w:
                        # plane-split final DMA: rows ship as they complete
                        for k0 in (0, 3, 6):
                            nc.sync.dma_start(
                                out=voutL[:, k0 : k0 + 3, dsl],
                                in_=otk[:, k0 : k0 + 3, :],
                            )
                    else:
                        nc.sync.dma_start(out=voutL[:, :, dsl], in_=otk)
                    b += cw

    _split_multi_waits(nc)
    return nc


# ----------------------------------------------------------------------------
# host-side execution
# ----------------------------------------------------------------------------
_CACHE = {}


def _get_runner():
    if "runner" in _CACHE:
        return _CACHE["runner"]
    import jax
    from jax.sharding import Mesh, PartitionSpec
    from jax.experimental.shard_map import shard_map
    from concourse.bass2jax import (
        _bass_exec_p,
        install_neuronx_cc_hook,
        partition_id_tensor,
    )

    nc = build_module()
    install_neuronx_cc_hook()
    partition_name = nc.partition_id_tensor.name if nc.partition_id_tensor else None
    in_names, out_names, out_avals, zero_outs = [], [], [], []
    for alloc in nc.m.functions[0].allocations:
        if not isinstance(alloc, mybir.MemoryLocationSet):
            continue
        name = alloc.memorylocations[0].name
        if alloc.kind == "ExternalInput":
            if name != partition_name:
                in_names.append(name)
        elif alloc.kind == "ExternalOutput":
            shape = tuple(alloc.tensor_shape)
            dtype = mybir.dt.np(alloc.dtype)
            out_names.append(name)
            out_avals.append(jax.core.ShapedArray(shape, dtype))
            zero_outs.append(np.zeros(shape, dtype))
    n_params = len(in_names)
    all_in_names = in_names + out_names + (
        [partition_name] if partition_name else []
    )

    def _body(*args):
        operands = list(args)
        if partition_name is not None:
            operands.append(partition_id_tensor())
        outs = _bass_exec_p.bind(
            *operands,
            out_avals=tuple(out_avals),
            in_names=tuple(all_in_names),
            out_names=tuple(out_names),
            lowering_input_output_aliases=(),
            sim_require_finite=True,
            sim_require_nnan=True,
            nc=nc,
        )
        return tuple(outs)

    devices = jax.devices()[:N_CORES]
    mesh = Mesh(np.asarray(devices), ("core",))
    n_outs = len(out_names)
    jf = jax.jit(
        shard_map(
            _body,
            mesh=mesh,
            in_specs=(PartitionSpec("core"),) * (n_params + n_outs),
            out_specs=(PartitionSpec("core"),) * n_outs,
            check_rep=False,
        ),
        donate_argnums=tuple(range(n_params, n_params + n_outs)),
        keep_unused=True,
    )
    _CACHE["runner"] = (jf, in_names, out_names, zero_outs)
    return _CACHE["runner"]


def kernel(trans, rotat, scal_dir, scal):
    jf, in_names, out_names, zero_outs = _get_runner()
    assert in_names == ["in9"], in_names

    # fp16 planar marshalling: planes (rx ry rz ux uy uz sx sy sz) per core
    a = np.empty((N_CORES, 9, E), dtype=np.float16)
    buf = np.ones((BPAD, 3), dtype=np.float16)
    for i, src in enumerate((rotat, scal_dir, scal)):
        buf[:B] = src
        if i > 0:
            buf[B:] = 1.0
        a[:, 3 * i : 3 * i + 3, :] = buf.reshape(N_CORES, E, 3).transpose(0, 2, 1)

    in9_host = a.reshape(N_CORES * 9, E)
    zeros = [
        np.zeros((N_CORES * z.shape[0], *z.shape[1:]), z.dtype) for z in zero_outs
    ]
    outs = jf(in9_host, *zeros)
    o = np.asarray(outs[0]).reshape(N_CORES, 9, E)
    m = o.transpose(0, 2, 1).reshape(BPAD, 9)[:B].astype(np.float32)
    out = np.empty((B, 3, 4), dtype=np.float32)
    out[:, :, :3] = m.reshape(B, 3, 3)
    out[:, :, 3] = trans
    return out


if __name__ == "__main__":
    rng = np.random.default_rng(0)
    ins = {
        "trans": rng.normal(size=(B, 3)).astype(np.float32),
        "rotat": rng.normal(size=(B, 3)).astype(np.float32),
        "scal_dir": rng.normal(size=(B, 3)).astype(np.float32),
        "scal": rng.normal(size=(B, 3)).astype(np.float32),
    }
    out = kernel(**ins)
    print(out.shape, out.dtype)


# revision 53
# speedup vs baseline: 1.0165x; 1.0165x over previous
"""AffCoeffToMatrix TRN2 kernel (v8: fp16 planar I/O, pipelined groups).

For each batch element (B = 2,000,000):
  R = rodrigues(rotat), U = rodrigues(scal_dir), D = exp(scal)
  M = R @ (U @ diag(D) @ U^T);  out = [M | trans]  -> [B, 3, 4] f32

Host marshals inputs to fp16 PLANAR layout (9 planes: r_xyz, u_xyz, s_xyz)
and reassembles the full [B,3,4] f32 output from the 9 fp16 M-planes the
device returns, inserting the trans column exactly.  Device HBM traffic is
36 B/elem (18 in + 18 out) vs 96 B/elem for f32 interleaved full I/O, and
all DMAs move >=984 B contiguous runs (full-rate).

On-core: L = 1968 elems/lane split into variable-width groups
(WIDTHS=(1,3,2,2) x F=246; narrow first group fills the pipeline fast).
Per group: ACT chain (squares, Ln/Exp in natural_log_exp table, Sin pair in
trig table) then per-246-wide-block builds.  Chain tiles with bufs=1 force
clean cross-group table phasing via WAR deps: exactly 2 table loads per
group.  cos(th/2) = sin(pi/2 - th/2) needs no range fold for th < 3pi.

Math per rotation (v = axis vector, th2 = |v|^2):
  lg = ln(th2); th = e^{lg/2}; rt = e^{-lg/2 + ln sqrt2} = sqrt2/th
  sh = sin(th/2), ch = cos(th/2); t2 = sh*rt; G = t2*v
  c2 = 1 - 2 sh^2 = cos th,  C2 = sqrt2*ch
  R = c2 I + G G^T + [C2 G]x          (G_i G_j = b v_i v_j, C2 G_k = a v_k)
Scaling: W = U diag(e^{s/2}), S = W W^T (6 unique), M = R @ S.

Engines (v1 cost model): ACT squares + transcendentals + most dG/sqW, DVE
fp16 2x tensor muls + 4x tensor-scalar coefficients, Pool assembly adds +
final M adds, SP all DMAs.  Drain: last block's M add and the final out
DMA are split per row-plane so the tail ships early.
"""
import math
import sys

for _p in ("/opt/trn_rl_repo", "/root/.axon_site/_ro/trn_rl_repo"):
    if _p not in sys.path:
        sys.path.append(_p)

import numpy as np

import concourse.bass as bass
import concourse.mybir as mybir
import concourse.tile as tile

F32 = mybir.dt.float32
F16 = mybir.dt.float16
AF = mybir.ActivationFunctionType
OP = mybir.AluOpType
PI = math.pi

# ---- hardcoded problem geometry ----
B = 2_000_000
N_CORES = 8
P = 128
F = 246            # base tile width (2F chunks = 984B >= 512B DMA full rate)
F2 = 2 * F         # group width (chains and builds)
NQ = 4             # groups per sweep
L = F2 * NQ        # elems per partition lane (1968)
E = P * L          # elems per core (251904)
BPAD = N_CORES * E


def _split_multi_waits(nc, limit=1, drain_limit=0):
    """This container's walrus cannot encode >1 sync-wait per instruction
    (Drain: none at all). Spill extras onto same-engine NOPs."""
    for b in nc.main_func.blocks:
        new = []
        for ins in b.instructions:
            si = getattr(ins, "sync_info", None)
            waits = list(si.on_wait) if (si is not None and si.on_wait) else []
            lim = drain_limit if isinstance(ins, mybir.InstDrain) else limit
            if len(waits) > lim:
                keep, spill = waits[:lim], waits[lim:]
                for w in spill:
                    nop = mybir.InstNoOp(
                        name=nc.get_next_instruction_name(),
                        sync_info=mybir.SyncInfo(on_wait=[w], on_update=[]),
                        bass_nofuse=True,
                        engine=ins.engine,
                    )
                    nc.register_instruction(nop)
                    new.append(nop)
                ins.sync_info = mybir.SyncInfo(
                    on_wait=keep, on_update=list(si.on_update or [])
                )
            new.append(ins)
        b.instructions[:] = new


def build_module(WIDTHS=(1, 3, 2, 2), CLUSTERS=None, DG_POOL=1, MS_DVE=1, SQW_DVE_EARLY=1, DG_DVE_EARLY=0, MP2_POOL_LATE=7, ROWSPLIT_FROM=7, S_DVE_LATE=4, OT_DVE_LATE=7, HALF_TAIL=False, TH2B=2, E3B=2, BLD=2, RU=3, PMAT_BUFS=2, PMS_BUFS=2, SMALL=2, OUT=2):
    nc = bass.Bass()
    in9 = nc.dram_tensor("in9", [9, E], F16, kind="ExternalInput")
    out9 = nc.dram_tensor("out9", [9, E], F16, kind="ExternalOutput")

    F1 = F
    assert sum(WIDTHS) == 8
    GQMAX = max(WIDTHS) * F1
    starts = [sum(WIDTHS[:i]) * F1 for i in range(len(WIDTHS))]

    vinL = in9[:].rearrange("k (p f) -> p k f", p=P)    # [P, 9, L]
    voutL = out9[:].rearrange("k (p f) -> p k f", p=P)  # [P, 9, L]

    with tile.TileContext(nc) as tc:
        with (
            tc.tile_pool(name="pin", bufs=2) as pin,      # in36 group
            tc.tile_pool(name="pth2", bufs=TH2B) as pth2, # th2 f32 (Pool writes)
            tc.tile_pool(name="ppsum", bufs=2, space="PSUM") as ppsum,  # lg f32
            tc.tile_pool(name="pch", bufs=1) as pch,      # chain transients (bufs=1 also forces cross-group ACT table phasing via WAR deps)
            tc.tile_pool(name="pcf", bufs=2) as pcf,      # t2, C2, c2 survivors
            tc.tile_pool(name="pe3", bufs=E3B) as pe3,    # e3
            tc.tile_pool(name="psq", bufs=1) as psq,      # squares scratch
            tc.tile_pool(name="pbld", bufs=BLD) as pbld,  # G/dG/av/p6 (F1 blocks)
            tc.tile_pool(name="pru", bufs=RU) as pru,     # RU18
            tc.tile_pool(name="pmat", bufs=PMAT_BUFS) as pmat,  # W9/sqW/S9/pp
            tc.tile_pool(name="pms", bufs=PMS_BUFS) as pms,     # ms
            tc.tile_pool(name="psmall", bufs=SMALL) as psmall,  # sdt/q3
            tc.tile_pool(name="pout", bufs=OUT) as pout,  # out per group
            tc.tile_pool(name="pc", bufs=1) as pc,
        ):
            # const bias tiles + dummy Ln to warm the natural_log_exp table
            # during the first DMA
            lnr2 = pc.tile([P, 1], F32, tag="lnr2")
            nc.vector.memset(lnr2[:], 0.5 * math.log(2.0))
            pi2 = pc.tile([P, 1], F32, tag="pi2")
            nc.vector.memset(pi2[:], PI / 2)
            warm1 = pc.tile([P, 1], F32, tag="warm1")
            nc.scalar.activation(warm1[:], pi2[:], AF.Ln)

            def chain_natlog(q, slot):
                w = WIDTHS[q]
                GQ = w * F1
                gsl = slice(starts[q], starts[q] + GQ)
                in36 = pin.tile([P, 9 * GQMAX], F16, tag=f"in36{slot}", name="in36")
                v36 = in36[:].rearrange("p (k f) -> p k f", k=9)[:, :, :GQ]
                if q <= 1:
                    # split early DMAs so squares start sooner
                    nc.sync.dma_start(out=v36[:, 0:6, :], in_=vinL[:, 0:6, gsl])
                    nc.sync.dma_start(out=v36[:, 6:9, :], in_=vinL[:, 6:9, gsl])
                else:
                    nc.sync.dma_start(out=v36, in_=vinL[:, :, gsl])

                def t16(pool, n, tag):
                    full = pool.tile([P, n * GQMAX], F16, tag=tag + str(slot), name=tag)
                    return full[:].rearrange("p (r f) -> p r f", r=n)[:, :, :GQ]

                th2f = pth2.tile([P, 2 * GQMAX], F32, tag="th2", name="th2")
                th2v = th2f[:].rearrange("p (r f) -> p r f", r=2)[:, :, :GQ]
                lgf = ppsum.tile([P, 2 * GQMAX], F32, tag="lg", name="lg")
                lgv = lgf[:].rearrange("p (r f) -> p r f", r=2)[:, :, :GQ]
                thv = t16(pch, 2, "th")
                rtv = t16(pch, 2, "rt")
                e3v = t16(pe3, 3, "e3")
                shv = t16(pch, 2, "sh")
                chv = t16(pch, 2, "ch")
                shsqv = t16(pch, 2, "shsq")
                t2v = t16(pcf, 2, "t2")
                C2v = t16(pcf, 2, "C2")
                c2v = t16(pcf, 2, "c2")

                def hs2(v, h):
                    return v[:, :, h * F1 : (h + 1) * F1]

                # squares + th2 + natlog phase: split per F1-half for
                # latency on the first two groups, full-width after (the
                # pipeline is full; fewer ACT instructions win)
                subs = [None] if (q > 1 and w <= 2) else range(w)
                for h in subs:
                    if h is None:
                        sl = slice(0, GQ)
                        wq = GQ
                    else:
                        sl = slice(h * F1, (h + 1) * F1)
                        wq = F1
                    sq = psq.tile([P, 6 * 2 * F1], F16, tag="sq", name="sq")
                    sqv = sq[:].rearrange("p (c f) -> p c f", c=6)[:, :, :wq]
                    nc.scalar.activation(sqv, v36[:, 0:6, sl], AF.Square)
                    tmp = psq.tile([P, 2 * 2 * F1], F16, tag="tmp", name="tmp")
                    tmpv = tmp[:].rearrange("p (r f) -> p r f", r=2)[:, :, :wq]
                    nc.gpsimd.tensor_add(tmpv, sqv[:, 0:4:3, :], sqv[:, 1:5:3, :])
                    nc.gpsimd.tensor_add(th2v[:, :, sl], tmpv, sqv[:, 2:6:3, :])
                    nc.scalar.activation(lgv[:, :, sl], th2v[:, :, sl], AF.Ln)
                    nc.scalar.activation(
                        thv[:, :, sl], lgv[:, :, sl], AF.Exp, scale=0.5
                    )
                    nc.scalar.activation(
                        rtv[:, :, sl], lgv[:, :, sl], AF.Exp, scale=-0.5,
                        bias=lnr2[:],
                    )
                    nc.scalar.activation(
                        e3v[:, :, sl], v36[:, 6:9, sl], AF.Exp, scale=0.5
                    )

                return {
                    "v36": v36, "th": thv, "rt": rtv, "e3": e3v,
                    "GQ": GQ,
                }

            def chain_trig(q, slot, stn):
                GQ = stn["GQ"]
                thv, rtv = stn["th"], stn["rt"]

                def t16(pool, n, tag):
                    full = pool.tile([P, n * GQMAX], F16, tag=tag + str(slot), name=tag)
                    return full[:].rearrange("p (r f) -> p r f", r=n)[:, :, :GQ]

                sh = t16(pch, 2, "sh")
                ch = t16(pch, 2, "ch")
                shsq = t16(pch, 2, "shsq")
                t2v = t16(pcf, 2, "t2")
                C2v = t16(pcf, 2, "C2")
                c2v = t16(pcf, 2, "c2")
                # cos(x) = sin(pi/2 - x): arg in [-1.44, pi/2] for th < 3pi,
                # always inside the Sin table domain -> no range fold needed.
                # Split per F1-half on group 1 so its first build starts
                # sooner (pch WAR deps still pin cross-group table phases).
                w = WIDTHS[q]
                tsubs = [None]
                for h in tsubs:
                    tsl = slice(0, GQ) if h is None else slice(h * F1, (h + 1) * F1)
                    nc.scalar.activation(
                        sh[:, :, tsl], thv[:, :, tsl], AF.Sin, scale=0.5
                    )
                    nc.scalar.activation(
                        ch[:, :, tsl], thv[:, :, tsl], AF.Sin, scale=-0.5,
                        bias=pi2[:],
                    )
                    nc.scalar.activation(
                        shsq[:, :, tsl], sh[:, :, tsl], AF.Square
                    )
                    nc.vector.tensor_mul(
                        t2v[:, :, tsl], sh[:, :, tsl], rtv[:, :, tsl]
                    )
                    nc.vector.tensor_scalar(
                        C2v[:, :, tsl], ch[:, :, tsl], math.sqrt(2.0), None,
                        OP.mult,
                    )
                    nc.vector.tensor_scalar(
                        c2v[:, :, tsl], shsq[:, :, tsl], -2.0, 1.0, OP.mult,
                        OP.add,
                    )
                return {
                    "v36": stn["v36"],
                    "t2": t2v,
                    "C2": C2v,
                    "c2": c2v,
                    "e3": stn["e3"],
                }

            BCTR = [0]

            def build(st, foff, bw, ot, osl, tagsfx=""):
                bidx = BCTR[0]
                BCTR[0] += 1
                """One bw-wide build block; writes M into ot[:, :, osl]."""
                sl = slice(foff, foff + bw)
                vv = st["v36"][:, 0:6, sl].rearrange("p (r c) f -> p r c f", r=2)
                t2s = st["t2"][:, :, sl]
                C2s = st["C2"][:, :, sl].unsqueeze(2)
                c2s = st["c2"][:, :, sl]
                e3s = st["e3"][:, :, sl]

                # G = t2 * v
                G = pbld.tile([P, 6 * bw], F16, tag=tagsfx + "G", name="G")
                Gv = G[:].rearrange("p (r c f) -> p r c f", r=2, c=3)
                nc.vector.tensor_mul(
                    Gv, t2s.unsqueeze(2).to_broadcast((P, 2, 3, bw)), vv
                )
                # dG = G^2 (ACT Square, or Pool mul on DG_POOL blocks)
                dG = pbld.tile([P, 6 * bw], F16, tag=tagsfx + "dG", name="dG")
                dGv = dG[:].rearrange("p (r c f) -> p r c f", r=2, c=3)
                if bidx < DG_DVE_EARLY:
                    nc.vector.tensor_mul(dGv, Gv, Gv)
                elif bidx < DG_DVE_EARLY + DG_POOL:
                    nc.gpsimd.tensor_mul(dGv, Gv, Gv)
                else:
                    nc.scalar.activation(dGv, Gv, AF.Square)
                # av planes in (z, x, y) order: av = C2 * G_perm
                av = pbld.tile([P, 6 * bw], F16, tag=tagsfx + "av", name="av")
                avv = av[:].rearrange("p (r c f) -> p r c f", r=2, c=3)
                nc.vector.tensor_mul(
                    avv[:, :, 0:1, :],
                    C2s.to_broadcast((P, 2, 1, bw)),
                    Gv[:, :, 2:3, :],
                )
                nc.vector.tensor_mul(
                    avv[:, :, 1:3, :],
                    C2s.to_broadcast((P, 2, 2, bw)),
                    Gv[:, :, 0:2, :],
                )
                # p6 = (G0G1, G1G2, G2G0)
                p6 = pbld.tile([P, 6 * bw], F16, tag=tagsfx + "p6", name="p6")
                p6v = p6[:].rearrange("p (r c f) -> p r c f", r=2, c=3)
                nc.vector.tensor_mul(
                    p6v[:, :, 0:2, :], Gv[:, :, 0:2, :], Gv[:, :, 1:3, :]
                )
                nc.vector.tensor_mul(
                    p6v[:, :, 2:3, :], Gv[:, :, 2:3, :], Gv[:, :, 0:1, :]
                )

                # RU18 assembly (Pool): R = c2 I + GG^T + [C2 G]x
                RU18 = pru.tile([P, 18 * bw], F16, tag=tagsfx + "RU18", name="RU18")
                ruv = RU18[:].rearrange("p (r k f) -> p r k f", r=2, k=9)
                c2b = c2s.unsqueeze(2).to_broadcast((P, 2, 3, bw))
                nc.gpsimd.tensor_add(ruv[:, :, 0:9:4, :], dGv, c2b)
                nc.gpsimd.tensor_add(
                    ruv[:, :, 3:8:4, :], p6v[:, :, 0:2, :], avv[:, :, 0:2, :]
                )
                nc.gpsimd.tensor_add(
                    ruv[:, :, 2, :], p6v[:, :, 2, :], avv[:, :, 2, :]
                )
                nc.gpsimd.tensor_sub(
                    ruv[:, :, 1:6:4, :], p6v[:, :, 0:2, :], avv[:, :, 0:2, :]
                )
                nc.gpsimd.tensor_sub(
                    ruv[:, :, 6, :], p6v[:, :, 2, :], avv[:, :, 2, :]
                )

                R9v = RU18[:, : 9 * bw].rearrange("p (k f) -> p k f", k=9)
                U9v = RU18[:, 9 * bw :].rearrange("p (i k f) -> p i k f", i=3, k=3)

                # W = U diag(e) (DVE), sqW (ACT)
                W9 = pmat.tile([P, 9 * bw], F16, tag=tagsfx + "W9", name="W9")
                W9v4 = W9[:].rearrange("p (i k f) -> p i k f", i=3, k=3)
                nc.vector.tensor_mul(
                    W9v4, U9v, e3s.unsqueeze(1).to_broadcast((P, 3, 3, bw))
                )
                sqW = pmat.tile([P, 9 * bw], F16, tag=tagsfx + "sqW", name="sqW")
                if bidx < SQW_DVE_EARLY:
                    nc.vector.tensor_mul(sqW[:], W9[:], W9[:])
                else:
                    nc.scalar.activation(sqW[:], W9[:], AF.Square)
                sqWv = sqW[:].rearrange("p (i k f) -> p i k f", i=3, k=3)

                # S unique-6: S00@0 S01@1 S02@2 S11@3 S12@5 S22@8
                S9 = pmat.tile([P, 9 * bw], F16, tag=tagsfx + "S9", name="S9")
                S9v = S9[:].rearrange("p (k f) -> p k f", k=9)
                sdt = psmall.tile([P, 3 * bw], F16, tag=tagsfx + "sdt", name="sdt")
                sdtv = sdt[:].rearrange("p (c f) -> p c f", c=3)
                nc.gpsimd.tensor_add(sdtv, sqWv[:, :, 0, :], sqWv[:, :, 1, :])
                nc.gpsimd.tensor_add(
                    S9v[:, 0:4:3, :], sdtv[:, 0:2, :], sqWv[:, 0:2, 2, :]
                )
                nc.gpsimd.tensor_add(S9v[:, 8, :], sdtv[:, 2, :], sqWv[:, 2, 2, :])
                # pp: row-pair products (01, 02, 12)
                pp = pmat.tile([P, 9 * bw], F16, tag=tagsfx + "pp", name="pp")
                ppv = pp[:].rearrange("p (g k f) -> p g k f", g=3, k=3)
                nc.vector.tensor_mul(
                    ppv[:, 0:2, :, :],
                    W9v4[:, 0, :, :].unsqueeze(1).to_broadcast((P, 2, 3, bw)),
                    W9v4[:, 1:3, :, :],
                )
                nc.vector.tensor_mul(
                    ppv[:, 2, :, :], W9v4[:, 1, :, :], W9v4[:, 2, :, :]
                )
                q3 = psmall.tile([P, 3 * bw], F16, tag=tagsfx + "q3", name="q3")
                q3v = q3[:].rearrange("p (g f) -> p g f", g=3)
                seng = nc.vector if bidx >= S_DVE_LATE else nc.gpsimd
                seng.tensor_add(q3v, ppv[:, :, 0, :], ppv[:, :, 1, :])
                seng.tensor_add(
                    S9v[:, 1:3, :], q3v[:, 0:2, :], ppv[:, 0:2, 2, :]
                )
                seng.tensor_add(S9v[:, 5, :], q3v[:, 2, :], ppv[:, 2, 2, :])

                # M = R @ S (DVE muls, Pool final add into out tile)
                srows = [S9v[:, 0:3, :], S9v[:, 1:7:2, :], S9v[:, 2:9:3, :]]
                otv = ot.rearrange("p (i j) f -> p i j f", i=3)[:, :, :, osl]

                def colb(k):
                    return (
                        R9v[:, k : k + 7 : 3, :]
                        .unsqueeze(2)
                        .to_broadcast((P, 3, 3, bw))
                    )

                def rowb(sr):
                    return sr.unsqueeze(1).to_broadcast((P, 3, 3, bw))

                mp1 = pmat.tile([P, 9 * bw], F16, tag=tagsfx + "pp", name="mp1")
                mp1v = mp1[:].rearrange("p (i j f) -> p i j f", i=3, j=3)
                nc.vector.tensor_mul(mp1v, colb(0), rowb(srows[0]))
                mp2 = pmat.tile([P, 9 * bw], F16, tag=tagsfx + "sqW", name="mp2")
                mp2v = mp2[:].rearrange("p (i j f) -> p i j f", i=3, j=3)
                (nc.gpsimd if bidx >= MP2_POOL_LATE else nc.vector).tensor_mul(
                    mp2v, colb(1), rowb(srows[1])
                )
                ms = pms.tile([P, 9 * bw], F16, tag=tagsfx + "ms", name="ms")
                msv = ms[:].rearrange("p (i j f) -> p i j f", i=3, j=3)
                (nc.vector if bidx >= 8 - MS_DVE else nc.gpsimd).tensor_add(
                    msv, mp1v, mp2v
                )
                mp3 = pmat.tile([P, 9 * bw], F16, tag=tagsfx + "W9", name="mp3")
                mp3v = mp3[:].rearrange("p (i j f) -> p i j f", i=3, j=3)
                nc.vector.tensor_mul(mp3v, colb(2), rowb(srows[2]))
                oeng = nc.vector if bidx >= OT_DVE_LATE else nc.gpsimd
                if bidx >= ROWSPLIT_FROM:
                    # drain: per-row adds so output planes ship early
                    for i in range(3):
                        oeng.tensor_add(
                            otv[:, i : i + 1, :, :],
                            msv[:, i : i + 1, :, :],
                            mp3v[:, i : i + 1, :, :],
                        )
                else:
                    oeng.tensor_add(otv, msv, mp3v)

            if CLUSTERS is None:
                CLUSTERS2 = tuple((i,) for i in range(len(WIDTHS)))
            else:
                CLUSTERS2 = CLUSTERS
            for cluster in CLUSTERS2:
              cl_states = {}
              for slot, q in enumerate(cluster):
                  cl_states[q] = chain_natlog(q, slot)
              for slot, q in enumerate(cluster):
                  cl_states[q] = chain_trig(q, slot, cl_states[q])
              for slot, q in enumerate(cluster):
                w = WIDTHS[q]
                GQ = w * F1
                gsl = slice(starts[q], starts[q] + GQ)
                st = cl_states[q]
                is_last_group = q == len(WIDTHS) - 1
                # fixed-size 2-block output chunks (keeps pout independent
                # of group width; odd tail chunk pays the <512B DMA rate)
                b = 0
                while b < w:
                    cw = min(2, w - b)
                    otf = pout.tile([P, 9 * 2 * F1], F16, tag="ot", name="ot")
                    otk = otf[:].rearrange("p (k f) -> p k f", k=9)[
                        :, :, : cw * F1
                    ]
                    for j in range(cw):
                        build(st, (b + j) * F1, F1, otk,
                              slice(j * F1, (j + 1) * F1))
                    dsl = slice(starts[q] + b * F1, starts[q] + (b + cw) * F1)
                    if is_last_group and b + cw == w:
                        # plane-split final DMA: rows ship as they complete
                        for k0 in (0, 3, 6):
                            nc.sync.dma_start(
                                out=voutL[:, k0 : k0 + 3, dsl],
                                in_=otk[:, k0 : k0 + 3, :],
                            )
                    else:
                        nc.sync.dma_start(out=voutL[:, :, dsl], in_=otk)
                    b += cw

    _split_multi_waits(nc)
    return nc


# ----------------------------------------------------------------------------
# host-side execution
# ----------------------------------------------------------------------------
_CACHE = {}


def _get_runner():
    if "runner" in _CACHE:
        return _CACHE["runner"]
    import jax
    from jax.sharding import Mesh, PartitionSpec
    from jax.experimental.shard_map import shard_map
    from concourse.bass2jax import (
        _bass_exec_p,
        install_neuronx_cc_hook,
        partition_id_tensor,
    )

    nc = build_module()
    install_neuronx_cc_hook()
    partition_name = nc.partition_id_tensor.name if nc.partition_id_tensor else None
    in_names, out_names, out_avals, zero_outs = [], [], [], []
    for alloc in nc.m.functions[0].allocations:
        if not isinstance(alloc, mybir.MemoryLocationSet):
            continue
        name = alloc.memorylocations[0].name
        if alloc.kind == "ExternalInput":
            if name != partition_name:
                in_names.append(name)
        elif alloc.kind == "ExternalOutput":
            shape = tuple(alloc.tensor_shape)
            dtype = mybir.dt.np(alloc.dtype)
            out_names.append(name)
            out_avals.append(jax.core.ShapedArray(shape, dtype))
            zero_outs.append(np.zeros(shape, dtype))
    n_params = len(in_names)
    all_in_names = in_names + out_names + (
        [partition_name] if partition_name else []
    )

    def _body(*args):
        operands = list(args)
        if partition_name is not None:
            operands.append(partition_id_tensor())
        outs = _bass_exec_p.bind(
            *operands,
            out_avals=tuple(out_avals),
            in_names=tuple(all_in_names),
            out_names=tuple(out_names),
            lowering_input_output_aliases=(),
            sim_require_finite=True,
            sim_require_nnan=True,
            nc=nc,
        )
        return tuple(outs)

    devices = jax.devices()[:N_CORES]
    mesh = Mesh(np.asarray(devices), ("core",))
    n_outs = len(out_names)
    jf = jax.jit(
        shard_map(
            _body,
            mesh=mesh,
            in_specs=(PartitionSpec("core"),) * (n_params + n_outs),
            out_specs=(PartitionSpec("core"),) * n_outs,
            check_rep=False,
        ),
        donate_argnums=tuple(range(n_params, n_params + n_outs)),
        keep_unused=True,
    )
    _CACHE["runner"] = (jf, in_names, out_names, zero_outs)
    return _CACHE["runner"]


def kernel(trans, rotat, scal_dir, scal):
    jf, in_names, out_names, zero_outs = _get_runner()
    assert in_names == ["in9"], in_names

    # fp16 planar marshalling: planes (rx ry rz ux uy uz sx sy sz) per core
    a = np.empty((N_CORES, 9, E), dtype=np.float16)
    buf = np.ones((BPAD, 3), dtype=np.float16)
    for i, src in enumerate((rotat, scal_dir, scal)):
        buf[:B] = src
        if i > 0:
            buf[B:] = 1.0
        a[:, 3 * i : 3 * i + 3, :] = buf.reshape(N_CORES, E, 3).transpose(0, 2, 1)

    in9_host = a.reshape(N_CORES * 9, E)
    zeros = [
        np.zeros((N_CORES * z.shape[0], *z.shape[1:]), z.dtype) for z in zero_outs
    ]
    outs = jf(in9_host, *zeros)
    o = np.asarray(outs[0]).reshape(N_CORES, 9, E)
    m = o.transpose(0, 2, 1).reshape(BPAD, 9)[:B].astype(np.float32)
    out = np.empty((B, 3, 4), dtype=np.float32)
    out[:, :, :3] = m.reshape(B, 3, 3)
    out[:, :, 3] = trans
    return out


if __name__ == "__main__":
    rng = np.random.default_rng(0)
    ins = {
        "trans": rng.normal(size=(B, 3)).astype(np.float32),
        "rotat": rng.normal(size=(B, 3)).astype(np.float32),
        "scal_dir": rng.normal(size=(B, 3)).astype(np.float32),
        "scal": rng.normal(size=(B, 3)).astype(np.float32),
    }
    out = kernel(**ins)
    print(out.shape, out.dtype)
